# revision 21
# baseline (speedup 1.0000x reference)
"""Trainium2 Bass kernel for Adapt_CSA — v4.

Reference computation (per sample, x: (C=256, H=64, W=64) f32):
  y    = mean(x, (H,W))                       # (C,)
  y'   = conv1d(y, w_c, SAME, k=5)            # (C,)
  yc   = sigmoid(x * y'[:, None, None])       # (C, H, W)
  avg  = mean(yc, C); mx = max(yc, C)         # (H, W) each
  sa   = sigmoid(conv2d([avg, mx], w_s, SAME))# (1, H, W)
  out  = yc * sa + x

Data parallel over batch: 32 samples -> 8 cores x 4 samples.
Channel-partition layout: 2 tiles of (128, 4096) per sample, bf16.

v4 vs v3 (143.9us): engine-balanced redesign from measured op costs.
- AR (channel max) cost is free-size-bound (~14.8us @ 4096 cols) and
  partition-count-independent -> no folds; Pool does ONLY the AR.
- GAP split between ACT (Copy+accum) and DVE (TS+accum) at a ratio
  that equalizes ACT ~= DVE ~= Pool ~= 14.5us/sample.
- staging DMAs merged 4->1 via partition-stride-32 APs (Sav, Sal).
- no windowed tail (v3 spent 16 extra DVE TTs on the last 2 samples).
- measured: AR 14.8, DVE TTmax 2.28, TS+acc 4.4, ACT sigmoid 3.78,
  ACT copy psum 1.1, matmul n=512 ~0.6 (pipelined), launch ~0.6.
"""

import sys

import numpy as np

sys.path.insert(0, "/opt/trn_rl_repo")

B, C, H, W = 32, 256, 64, 64
HW = H * W  # 4096
N_CORES = 8
SPC = B // N_CORES  # samples per core = 4
PAD = 68  # 64 + 2*2 zero border for SAME 5x5 conv
PADHW = PAD * PAD  # 4624
WIN = 63 * PAD + 64  # 4348 window length per patch row
GA = 2624  # GAP pixels handled by ACT per tile; DVE takes the rest (1472)

_cache = {}


def _build_graph():
    import concourse.bass as bass
    import concourse.bacc as bacc
    import concourse.tile as tile
    from concourse import bass_isa, library_config, mybir

    f32 = mybir.dt.float32
    bf16 = mybir.dt.bfloat16
    AF = mybir.ActivationFunctionType
    ALU = mybir.AluOpType

    nc = bacc.Bacc("TRN2", target_bir_lowering=False)

    x_ext = nc.declare_dram_parameter("x", [SPC, 2, 128, HW], bf16, isOutput=False)
    bmat_ext = nc.declare_dram_parameter("bmat", [2, 128, 256], f32, isOutput=False)
    ws_ext = nc.declare_dram_parameter("ws", [50, 1], f32, isOutput=False)
    out_ext = nc.declare_dram_parameter("out", [SPC, 2, 128, HW], bf16, isOutput=True)

    # double-buffered padded avg/max maps + sa staging in DRAM
    pads_b = [nc.dram_tensor(f"pads{j}", [2, PAD, PAD], bf16) for j in range(2)]
    salin_b = [nc.dram_tensor(f"salin{j}", [HW], bf16) for j in range(2)]

    with tile.TileContext(nc) as tc:
        with (
            tc.tile_pool(name="singles", bufs=1) as singles,
            tc.tile_pool(name="px", bufs=5) as px,
            tc.tile_pool(name="pyc", bufs=4) as pyc,
            tc.tile_pool(name="pjunk", bufs=1) as pjunk,
            tc.tile_pool(name="ppm", bufs=2) as ppm,
            tc.tile_pool(name="ppatch", bufs=1) as ppatch,
            tc.tile_pool(name="psar", bufs=1) as psar,
            tc.tile_pool(name="psab", bufs=2) as psab,
            tc.tile_pool(name="pmst", bufs=1) as pmst,
            tc.tile_pool(name="small", bufs=4) as small,
            tc.tile_pool(name="ps_y", bufs=1, space="PSUM") as ps_y,
            tc.tile_pool(name="ps_mean", bufs=2, space="PSUM") as ps_mean,
            tc.tile_pool(name="ps_sa", bufs=1, space="PSUM") as ps_sa,
        ):
            nc.gpsimd.load_library(library_config.attn)

            # ---- constants ----
            bmat_sb = singles.tile([128, 2, 256], f32)
            nc.scalar.dma_start(
                out=bmat_sb, in_=bmat_ext[:].rearrange("t p m -> p t m")
            )
            ws_f32 = singles.tile([50, 1], f32)
            nc.scalar.dma_start(out=ws_f32, in_=ws_ext[:])
            ws_bf = singles.tile([50, 1], bf16)
            nc.vector.tensor_copy(out=ws_bf, in_=ws_f32)
            ones_bf = singles.tile([128, 1], bf16)
            nc.vector.memset(ones_bf, 1.0)
            # zero both DRAM pad buffers once (borders stay zero)
            zero68 = singles.tile([PAD, 2, PAD], bf16)
            nc.vector.memset(zero68, 0.0)
            for j in range(2):
                nc.gpsimd.dma_start(
                    out=pads_b[j][:].rearrange("c h w -> h c w"), in_=zero68
                )
            junk0 = pjunk.tile([128, GA], bf16)
            junk1 = pjunk.tile([128, HW - GA], bf16)

            st = [dict() for _ in range(SPC)]

            # ---- stages ----
            def stage_L(s):
                x_t = px.tile([128, 2, HW], bf16, tag="x")
                for t in range(2):
                    nc.sync.dma_start(out=x_t[:, t], in_=x_ext[s, t])
                st[s].update(x_t=x_t)

            def stage_Ga(s):
                # GAP front part on ACT (Copy pass with accum)
                x_t = st[s]["x_t"]
                ysum = small.tile([128, 4], f32, tag="ysum")
                for t in range(2):
                    nc.scalar.activation(
                        out=junk0,
                        in_=x_t[:, t, 0:GA],
                        func=AF.Copy,
                        bias=0.0,
                        scale=1.0,
                        accum_out=ysum[:, t : t + 1],
                    )
                st[s].update(ysum=ysum)

            def stage_Gv(s):
                # GAP tail part on DVE (tensor_scalar with accum)
                x_t, ysum = st[s]["x_t"], st[s]["ysum"]
                for t in range(2):
                    nc.vector.tensor_scalar(
                        out=junk1,
                        in0=x_t[:, t, GA:HW],
                        scalar1=1.0,
                        scalar2=0.0,
                        op0=ALU.mult,
                        op1=ALU.add,
                        accum_out=ysum[:, 2 + t : 3 + t],
                    )

            def stage_E(s):
                # ysum cols: 0=ACT t0, 1=ACT t1, 2=DVE t0, 3=DVE t1; the
                # half-sums merge inside the accumulating chconv matmuls
                x_t, ysum = st[s]["x_t"], st[s]["ysum"]
                py_t = ps_y.tile([128, 2], f32, tag="py")
                for mt in range(2):
                    for j in range(4):
                        nc.tensor.matmul(
                            py_t[:, mt : mt + 1],
                            lhsT=bmat_sb[:, j % 2, mt * 128 : (mt + 1) * 128],
                            rhs=ysum[:, j : j + 1],
                            start=(j == 0),
                            stop=(j == 3),
                        )
                yscale = small.tile([128, 2], f32, tag="yscale")
                nc.vector.tensor_copy(out=yscale, in_=py_t)
                yc = pyc.tile([128, 2, HW], bf16, tag="yc")
                # column-half order (h0-t0, h0-t1, h1-t0, h1-t1) so the
                # first premax half can start after two quarter-sigmoids
                for h in range(2):
                    for t in range(2):
                        nc.scalar.activation(
                            out=yc[:, t, 2048 * h : 2048 * (h + 1)],
                            in_=x_t[:, t, 2048 * h : 2048 * (h + 1)],
                            func=AF.Sigmoid,
                            scale=yscale[:, t : t + 1],
                        )
                st[s].update(yc=yc)

            def stage_M(s):
                yc = st[s]["yc"]
                pm = ppm.tile([128, HW], bf16, tag="pm")
                mid = 2176 if s >= SPC - 2 else 2048
                for c0, c1 in ((0, mid), (mid, HW)):
                    cs = slice(c0, c1)
                    nc.vector.tensor_max(
                        out=pm[:, cs], in0=yc[:, 0, cs], in1=yc[:, 1, cs]
                    )
                st[s].update(pm=pm)

            def stage_A(s):
                pbuf = s % 2
                pm = st[s]["pm"]
                mid = 2176 if s >= SPC - 2 else 2048
                for c0, c1 in ((0, mid), (mid, HW)):
                    nc.gpsimd.partition_all_reduce(
                        pm[:, c0:c1], pm[:, c0:c1], 128, bass_isa.ReduceOp.max
                    )
                    nc.gpsimd.dma_start(
                        out=pads_b[pbuf][
                            1, 2 + c0 // 64 : 2 + c1 // 64, 2:66
                        ],
                        in_=pm[0:1, c0:c1].rearrange(
                            "p (h w) -> p h w", h=(c1 - c0) // 64
                        ),
                    )

            def stage_Me(s):
                yc = st[s]["yc"]
                pmean = ps_mean.tile([128, 1024], f32, tag="mean")
                for k in range(4):
                    for h in range(2):
                        c0 = 1024 * k + 512 * h
                        for t in range(2):
                            nc.tensor.matmul(
                                pmean[32 * k : 32 * k + 1, 512 * h : 512 * (h + 1)],
                                lhsT=ones_bf,
                                rhs=yc[:, t, c0 : c0 + 512],
                                start=(t == 0),
                                stop=(t == 1),
                                tile_position=(0, 32 * k),
                            )
                st[s].update(pmean=pmean)

            def stage_Mc(s):
                pbuf = s % 2
                pmean = st[s]["pmean"]
                mstage = pmst.tile([128, 1024], bf16, tag="mstage")
                nc.scalar.copy(out=mstage, in_=pmean)
                # one DMA: partitions {0,32,64,96} x (16 rows x 64 px) -> pads
                # (scalar queue: right behind the mstage copy, and off the
                # store-congested sync queue in the tail)
                nc.scalar.dma_start(
                    out=pads_b[pbuf][0, 2:66, 2:66].rearrange(
                        "(q hh) w -> q hh w", q=4
                    ),
                    in_=mstage[0:97:32].rearrange("q (hh w) -> q hh w", hh=16),
                )

            def stage_P(s, half=None):
                # half=0: out rows 0:32 (windows at pads rows 0:36)
                # half=1: out rows 32:64 (windows at pads rows 32:66)
                pbuf = s % 2
                if half is None:
                    patches = ppatch.tile([50, 4352], bf16, tag="patch")
                    win, roff = WIN, 0
                else:
                    patches = ppatch.tile([50, 2176], bf16, tag=f"patch{half}")
                    win, roff = 31 * PAD + 64, 32 * half
                for c in range(2):
                    src = bass.AP(
                        tensor=pads_b[pbuf],
                        offset=c * PADHW + roff * PAD,
                        ap=[[PAD, 5], [1, 5], [1, win]],
                    )
                    nc.sync.dma_start(
                        out=patches[25 * c : 25 * (c + 1), 0:win], in_=src
                    )
                st[s][f"patches{half}" if half is not None else "patches"] = patches

            def stage_Cv(s, half=None):
                if half is None:
                    patches = st[s]["patches"]
                    ks = range(4)
                    psa = ps_sa.tile([128, 1088], f32, tag="sa")
                    st[s]["psa"] = psa
                else:
                    patches = st[s][f"patches{half}"]
                    if half == 0:
                        psa = ps_sa.tile([128, 1088], f32, tag="sa")
                        st[s]["psa"] = psa
                    else:
                        psa = st[s]["psa"]
                    ks = range(2 * half, 2 * half + 2)
                for k in ks:
                    kk = k if half is None else k - 2 * half
                    for c0, nn in ((0, 512), (512, 512), (1024, 64)):
                        nc.tensor.matmul(
                            psa[32 * k : 32 * k + 1, c0 : c0 + nn],
                            lhsT=ws_bf,
                            rhs=patches[:, 1088 * kk + c0 : 1088 * kk + c0 + nn],
                            start=True,
                            stop=True,
                            tile_position=(0, 32 * k),
                        )
                if half is None:
                    sa_row = psar.tile([128, 1088], bf16, tag="sarow")
                    nc.scalar.activation(out=sa_row, in_=psa, func=AF.Sigmoid)
                    st[s].update(sa_row=sa_row)
                else:
                    if half == 0:
                        sa_row = psar.tile([128, 1088], bf16, tag="sarow")
                        st[s].update(sa_row=sa_row)
                    else:
                        sa_row = st[s]["sa_row"]
                    nc.scalar.activation(
                        out=sa_row[64 * half : 64 * half + 33],
                        in_=psa[64 * half : 64 * half + 33],
                        func=AF.Sigmoid,
                    )

            def stage_Sl(s, half=None):
                pbuf = s % 2
                sa_row = st[s]["sa_row"]
                if half is not None:
                    q0 = 64 * half
                    nc.scalar.dma_start(
                        out=salin_b[pbuf][2048 * half : 2048 * (half + 1)].rearrange(
                            "(q hh w) -> q hh w", q=2, hh=16
                        ),
                        in_=sa_row[q0 : q0 + 33 : 32].rearrange(
                            "q (hh w) -> q hh w", hh=16
                        )[:, :, 0:64],
                    )
                    if half == 0:
                        sab = psab.tile([128, HW], bf16, tag="sab")
                        st[s].update(sab=sab)
                    else:
                        sab = st[s]["sab"]
                    nc.scalar.dma_start(
                        out=sab[:, 2048 * half : 2048 * (half + 1)],
                        in_=bass.AP(
                            tensor=salin_b[pbuf],
                            offset=2048 * half,
                            ap=[[0, 128], [1, 2048]],
                        ),
                    )
                    return
                # one DMA: rows {0,32,64,96} 68-pitch -> linear DRAM staging
                # (scalar queue: keeps the fuse-critical salin+bcast chain
                # off the store-congested sync queue in the tail phase)
                nc.scalar.dma_start(
                    out=salin_b[pbuf][:].rearrange("(q hh w) -> q hh w", q=4, hh=16),
                    in_=sa_row[0:97:32].rearrange("q (hh w) -> q hh w", hh=16)[
                        :, :, 0:64
                    ],
                )
                sab = psab.tile([128, HW], bf16, tag="sab")
                nc.scalar.dma_start(
                    out=sab,
                    in_=bass.AP(
                        tensor=salin_b[pbuf], offset=0, ap=[[0, 128], [1, HW]]
                    ),
                )
                st[s].update(sab=sab)

            def stage_T(s, half=None):
                x_t, yc, sab = st[s]["x_t"], st[s]["yc"], st[s]["sab"]
                # last sample's stores ride the tail-idle scalar queue
                oeng = nc.scalar if s == SPC - 1 else nc.sync
                cs = (
                    slice(0, HW)
                    if half is None
                    else slice(2048 * half, 2048 * (half + 1))
                )
                for t in range(2):
                    nc.vector.tensor_mul(
                        out=yc[:, t, cs], in0=yc[:, t, cs], in1=sab[:, cs]
                    )
                    nc.vector.tensor_add(
                        out=x_t[:, t, cs], in0=yc[:, t, cs], in1=x_t[:, t, cs]
                    )
                    oeng.dma_start(out=out_ext[s, t, :, cs], in_=x_t[:, t, cs])

            # Emission order per step, tuned per engine queue:
            # ACT: sigmoids(s-1) first, GAP(s) late, mstage(s-2)/saσ(s-3) fill
            # DVE: yscale(s-1), premax(s-2) [feeds Pool], GAP(s), fuse(s-4)
            # PE:  chconv(s-1), conv(s-3), mean(s-2)
            # SP:  loads(s), patches(s-3), salin/bcast(s-3), AR-row/avg(s-2),
            #      stores(s-4)
            stages = (
                (0, stage_L),
                (1, stage_E),
                (3, stage_P),
                (3, stage_Cv),
                (2, stage_M),
                (0, stage_Ga),
                (0, stage_Gv),
                (2, stage_A),
                (2, stage_Me),
                (4, stage_T),
                (3, stage_Sl),
                (2, stage_Mc),
            )
            split_stages = {stage_P, stage_Cv, stage_Sl, stage_T}
            for step in range(SPC + 4):
                for d, fn in stages:
                    s = step - d
                    if 0 <= s < SPC:
                        if s >= SPC - 2 and fn in split_stages:
                            fn(s, half=0)
                            fn(s, half=1)
                        else:
                            fn(s)

    nc.compile()
    return nc


def _prep_inputs(x, w_c, w_s):
    """Shard + build per-core input maps (host side, cheap)."""
    import ml_dtypes

    wc = np.asarray(w_c, dtype=np.float32).reshape(5)
    ws4 = np.asarray(w_s, dtype=np.float32).reshape(2, 5, 5)

    # banded matrix: y'[m] = sum_k y[k] * wc[k - m + 2];  GAP 1/4096 folded in
    k = np.arange(C)[:, None]
    m = np.arange(C)[None, :]
    d = k - m + 2
    bmat = np.where((d >= 0) & (d < 5), wc[np.clip(d, 0, 4)], 0.0).astype(np.float32)
    bmat = (bmat / HW).reshape(2, 128, 256)

    # conv weights vector, rows = c*25 + ky*5 + kx ; channel-mean 1/256 folded in
    wsv = ws4.copy()
    wsv[0] /= C
    wsv = wsv.reshape(50, 1).astype(np.float32)

    xs = np.asarray(x, dtype=np.float32).astype(ml_dtypes.bfloat16).reshape(
        N_CORES, SPC, 2, 128, HW
    )
    in_maps = [{"x": xs[i], "bmat": bmat, "ws": wsv} for i in range(N_CORES)]
    return in_maps


def run(x, w_c, w_s, trace=False):
    from concourse.bass_utils import run_bass_kernel_spmd

    if "nc" not in _cache:
        _cache["nc"] = _build_graph()
    nc = _cache["nc"]
    in_maps = _prep_inputs(x, w_c, w_s)
    res = run_bass_kernel_spmd(
        nc, in_maps, core_ids=list(range(N_CORES)), trace=trace
    )
    out = np.concatenate(
        [
            res.results[i]["out"].astype(np.float32).reshape(SPC, C, H, W)
            for i in range(N_CORES)
        ],
        axis=0,
    )
    return out, res


def kernel(x, w_c, w_s):
    out, _ = run(x, w_c, w_s, trace=False)
    return out.astype(np.float32)


# revision 24
# speedup vs baseline: 1.0590x; 1.0590x over previous
"""Trainium2 Bass kernel for Adapt_CSA — v4.

Reference computation (per sample, x: (C=256, H=64, W=64) f32):
  y    = mean(x, (H,W))                       # (C,)
  y'   = conv1d(y, w_c, SAME, k=5)            # (C,)
  yc   = sigmoid(x * y'[:, None, None])       # (C, H, W)
  avg  = mean(yc, C); mx = max(yc, C)         # (H, W) each
  sa   = sigmoid(conv2d([avg, mx], w_s, SAME))# (1, H, W)
  out  = yc * sa + x

Data parallel over batch: 32 samples -> 8 cores x 4 samples.
Channel-partition layout: 2 tiles of (128, 4096) per sample, bf16.

v4 vs v3 (143.9us): engine-balanced redesign from measured op costs.
- AR (channel max) cost is free-size-bound (~14.8us @ 4096 cols) and
  partition-count-independent -> no folds; Pool does ONLY the AR.
- GAP split between ACT (Copy+accum) and DVE (TS+accum) at a ratio
  that equalizes ACT ~= DVE ~= Pool ~= 14.5us/sample.
- staging DMAs merged 4->1 via partition-stride-32 APs (Sav, Sal).
- no windowed tail (v3 spent 16 extra DVE TTs on the last 2 samples).
- measured: AR 14.8, DVE TTmax 2.28, TS+acc 4.4, ACT sigmoid 3.78,
  ACT copy psum 1.1, matmul n=512 ~0.6 (pipelined), launch ~0.6.
"""

import sys

import numpy as np

sys.path.insert(0, "/opt/trn_rl_repo")

B, C, H, W = 32, 256, 64, 64
HW = H * W  # 4096
N_CORES = 8
SPC = B // N_CORES  # samples per core = 4
PAD = 68  # 64 + 2*2 zero border for SAME 5x5 conv
PADHW = PAD * PAD  # 4624
WIN = 63 * PAD + 64  # 4348 window length per patch row
GA = 2624  # GAP pixels handled by ACT per tile; DVE takes the rest (1472)

_cache = {}


def _build_graph():
    import concourse.bass as bass
    import concourse.bacc as bacc
    import concourse.tile as tile
    from concourse import bass_isa, library_config, mybir

    f32 = mybir.dt.float32
    bf16 = mybir.dt.bfloat16
    AF = mybir.ActivationFunctionType
    ALU = mybir.AluOpType

    nc = bacc.Bacc("TRN2", target_bir_lowering=False)

    x_ext = nc.declare_dram_parameter("x", [SPC, 2, 128, HW], bf16, isOutput=False)
    bmat_ext = nc.declare_dram_parameter("bmat", [2, 128, 256], f32, isOutput=False)
    ws_ext = nc.declare_dram_parameter("ws", [50, 1], f32, isOutput=False)
    out_ext = nc.declare_dram_parameter("out", [SPC, 2, 128, HW], bf16, isOutput=True)

    # double-buffered padded avg/max maps + sa staging in DRAM
    pads_b = [nc.dram_tensor(f"pads{j}", [2, PAD, PAD], bf16) for j in range(2)]
    salin_b = [nc.dram_tensor(f"salin{j}", [HW], bf16) for j in range(2)]

    with tile.TileContext(nc) as tc:
        with (
            tc.tile_pool(name="singles", bufs=1) as singles,
            tc.tile_pool(name="px", bufs=4) as px,
            tc.tile_pool(name="pyc", bufs=4) as pyc,
            tc.tile_pool(name="pjunk", bufs=1) as pjunk,
            tc.tile_pool(name="ppm", bufs=2) as ppm,
            tc.tile_pool(name="ppatch", bufs=1) as ppatch,
            tc.tile_pool(name="psar", bufs=1) as psar,
            tc.tile_pool(name="psab", bufs=2) as psab,
            tc.tile_pool(name="psabh", bufs=1) as psabh,
            tc.tile_pool(name="pmst", bufs=1) as pmst,
            tc.tile_pool(name="small", bufs=4) as small,
            tc.tile_pool(name="ps_y", bufs=1, space="PSUM") as ps_y,
            tc.tile_pool(name="ps_mean", bufs=2, space="PSUM") as ps_mean,
            tc.tile_pool(name="ps_sa", bufs=1, space="PSUM") as ps_sa,
        ):
            nc.gpsimd.load_library(library_config.attn)

            # ---- constants ----
            bmat_sb = singles.tile([128, 2, 256], f32)
            nc.scalar.dma_start(
                out=bmat_sb, in_=bmat_ext[:].rearrange("t p m -> p t m")
            )
            ws_f32 = singles.tile([50, 1], f32)
            nc.scalar.dma_start(out=ws_f32, in_=ws_ext[:])
            ws_bf = singles.tile([50, 1], bf16)
            nc.vector.tensor_copy(out=ws_bf, in_=ws_f32)
            ones_bf = singles.tile([128, 1], bf16)
            nc.vector.memset(ones_bf, 1.0)
            # zero both DRAM pad buffers once (borders stay zero)
            zero68 = singles.tile([PAD, 2, PAD], bf16)
            nc.vector.memset(zero68, 0.0)
            for j in range(2):
                nc.gpsimd.dma_start(
                    out=pads_b[j][:].rearrange("c h w -> h c w"), in_=zero68
                )
            junk0 = pjunk.tile([128, GA], bf16)
            junk1 = pjunk.tile([128, HW - GA], bf16)

            st = [dict() for _ in range(SPC)]

            # ---- stages ----
            def stage_L(s):
                x_t = px.tile([128, 2, HW], bf16, tag="x")
                for t in range(2):
                    nc.sync.dma_start(out=x_t[:, t], in_=x_ext[s, t])
                st[s].update(x_t=x_t)

            def stage_Ga(s):
                # GAP front part on ACT (Copy pass with accum)
                x_t = st[s]["x_t"]
                ysum = small.tile([128, 4], f32, tag="ysum")
                for t in range(2):
                    nc.scalar.activation(
                        out=junk0,
                        in_=x_t[:, t, 0:GA],
                        func=AF.Copy,
                        bias=0.0,
                        scale=1.0,
                        accum_out=ysum[:, t : t + 1],
                    )
                st[s].update(ysum=ysum)

            def stage_Gv(s):
                # GAP tail part on DVE (tensor_scalar with accum)
                x_t, ysum = st[s]["x_t"], st[s]["ysum"]
                for t in range(2):
                    nc.vector.tensor_scalar(
                        out=junk1,
                        in0=x_t[:, t, GA:HW],
                        scalar1=1.0,
                        scalar2=0.0,
                        op0=ALU.mult,
                        op1=ALU.add,
                        accum_out=ysum[:, 2 + t : 3 + t],
                    )

            def stage_E(s):
                # ysum cols: 0=ACT t0, 1=ACT t1, 2=DVE t0, 3=DVE t1; the
                # half-sums merge inside the accumulating chconv matmuls
                x_t, ysum = st[s]["x_t"], st[s]["ysum"]
                py_t = ps_y.tile([128, 2], f32, tag="py")
                for mt in range(2):
                    for j in range(4):
                        nc.tensor.matmul(
                            py_t[:, mt : mt + 1],
                            lhsT=bmat_sb[:, j % 2, mt * 128 : (mt + 1) * 128],
                            rhs=ysum[:, j : j + 1],
                            start=(j == 0),
                            stop=(j == 3),
                        )
                yscale = small.tile([128, 2], f32, tag="yscale")
                nc.vector.tensor_copy(out=yscale, in_=py_t)
                yc = pyc.tile([128, 2, HW], bf16, tag="yc")
                # column-half order (h0-t0, h0-t1, h1-t0, h1-t1) so the
                # first premax half can start after two quarter-sigmoids
                for h in range(2):
                    for t in range(2):
                        nc.scalar.activation(
                            out=yc[:, t, 2048 * h : 2048 * (h + 1)],
                            in_=x_t[:, t, 2048 * h : 2048 * (h + 1)],
                            func=AF.Sigmoid,
                            scale=yscale[:, t : t + 1],
                        )
                st[s].update(yc=yc)

            def stage_M(s):
                yc = st[s]["yc"]
                pm = ppm.tile([128, HW], bf16, tag="pm")
                mid = 2176 if s == SPC - 1 else 2048
                for c0, c1 in ((0, mid), (mid, HW)):
                    cs = slice(c0, c1)
                    nc.vector.tensor_max(
                        out=pm[:, cs], in0=yc[:, 0, cs], in1=yc[:, 1, cs]
                    )
                st[s].update(pm=pm)

            def stage_A(s):
                pbuf = s % 2
                pm = st[s]["pm"]
                mid = 2176 if s == SPC - 1 else 2048
                for c0, c1 in ((0, mid), (mid, HW)):
                    nc.gpsimd.partition_all_reduce(
                        pm[:, c0:c1], pm[:, c0:c1], 128, bass_isa.ReduceOp.max
                    )
                    nc.gpsimd.dma_start(
                        out=pads_b[pbuf][
                            1, 2 + c0 // 64 : 2 + c1 // 64, 2:66
                        ],
                        in_=pm[0:1, c0:c1].rearrange(
                            "p (h w) -> p h w", h=(c1 - c0) // 64
                        ),
                    )

            def stage_Me(s):
                yc = st[s]["yc"]
                pmean = ps_mean.tile([128, 1024], f32, tag="mean")
                for k in range(4):
                    for h in range(2):
                        c0 = 1024 * k + 512 * h
                        for t in range(2):
                            nc.tensor.matmul(
                                pmean[32 * k : 32 * k + 1, 512 * h : 512 * (h + 1)],
                                lhsT=ones_bf,
                                rhs=yc[:, t, c0 : c0 + 512],
                                start=(t == 0),
                                stop=(t == 1),
                                tile_position=(0, 32 * k),
                            )
                st[s].update(pmean=pmean)

            def stage_Mc(s):
                pbuf = s % 2
                pmean = st[s]["pmean"]
                mstage = pmst.tile([128, 1024], bf16, tag="mstage")
                nc.scalar.copy(out=mstage, in_=pmean)
                # one DMA: partitions {0,32,64,96} x (16 rows x 64 px) -> pads
                # (scalar queue: right behind the mstage copy, and off the
                # store-congested sync queue in the tail)
                nc.scalar.dma_start(
                    out=pads_b[pbuf][0, 2:66, 2:66].rearrange(
                        "(q hh) w -> q hh w", q=4
                    ),
                    in_=mstage[0:97:32].rearrange("q (hh w) -> q hh w", hh=16),
                )

            def stage_P(s, half=None):
                # half=0: out rows 0:32 (windows at pads rows 0:36)
                # half=1: out rows 32:64 (windows at pads rows 32:66)
                pbuf = s % 2
                if half is None:
                    patches = ppatch.tile([50, 4352], bf16, tag="patch")
                    win, roff = WIN, 0
                else:
                    patches = ppatch.tile([50, 2176], bf16, tag=f"patch{half}")
                    win, roff = 31 * PAD + 64, 32 * half
                for c in range(2):
                    src = bass.AP(
                        tensor=pads_b[pbuf],
                        offset=c * PADHW + roff * PAD,
                        ap=[[PAD, 5], [1, 5], [1, win]],
                    )
                    nc.sync.dma_start(
                        out=patches[25 * c : 25 * (c + 1), 0:win], in_=src
                    )
                st[s][f"patches{half}" if half is not None else "patches"] = patches

            def stage_Cv(s, half=None):
                if half is None:
                    patches = st[s]["patches"]
                    ks = range(4)
                    psa = ps_sa.tile([128, 1088], f32, tag="sa")
                    st[s]["psa"] = psa
                else:
                    patches = st[s][f"patches{half}"]
                    if half == 0:
                        psa = ps_sa.tile([128, 1088], f32, tag="sa")
                        st[s]["psa"] = psa
                    else:
                        psa = st[s]["psa"]
                    ks = range(2 * half, 2 * half + 2)
                for k in ks:
                    kk = k if half is None else k - 2 * half
                    for c0, nn in ((0, 512), (512, 512), (1024, 64)):
                        nc.tensor.matmul(
                            psa[32 * k : 32 * k + 1, c0 : c0 + nn],
                            lhsT=ws_bf,
                            rhs=patches[:, 1088 * kk + c0 : 1088 * kk + c0 + nn],
                            start=True,
                            stop=True,
                            tile_position=(0, 32 * k),
                        )
                if half is None:
                    sa_row = psar.tile([128, 1088], bf16, tag="sarow")
                    nc.scalar.activation(out=sa_row, in_=psa, func=AF.Sigmoid)
                    st[s].update(sa_row=sa_row)
                else:
                    if half == 0:
                        sa_row = psar.tile([128, 1088], bf16, tag="sarow")
                        st[s].update(sa_row=sa_row)
                    else:
                        sa_row = st[s]["sa_row"]
                    nc.scalar.activation(
                        out=sa_row[64 * half : 64 * half + 33],
                        in_=psa[64 * half : 64 * half + 33],
                        func=AF.Sigmoid,
                    )

            def stage_Sl(s, half=None):
                pbuf = s % 2
                sa_row = st[s]["sa_row"]
                if half is not None:
                    q0 = 64 * half
                    nc.scalar.dma_start(
                        out=salin_b[pbuf][2048 * half : 2048 * (half + 1)].rearrange(
                            "(q hh w) -> q hh w", q=2, hh=16
                        ),
                        in_=sa_row[q0 : q0 + 33 : 32].rearrange(
                            "q (hh w) -> q hh w", hh=16
                        )[:, :, 0:64],
                    )
                    sabh = psabh.tile([128, 2048], bf16, tag=f"sab{half}")
                    st[s][f"sab{half}"] = sabh
                    nc.scalar.dma_start(
                        out=sabh,
                        in_=bass.AP(
                            tensor=salin_b[pbuf],
                            offset=2048 * half,
                            ap=[[0, 128], [1, 2048]],
                        ),
                    )
                    return
                # one DMA: rows {0,32,64,96} 68-pitch -> linear DRAM staging
                # (scalar queue: keeps the fuse-critical salin+bcast chain
                # off the store-congested sync queue in the tail phase)
                nc.scalar.dma_start(
                    out=salin_b[pbuf][:].rearrange("(q hh w) -> q hh w", q=4, hh=16),
                    in_=sa_row[0:97:32].rearrange("q (hh w) -> q hh w", hh=16)[
                        :, :, 0:64
                    ],
                )
                sab = psab.tile([128, HW], bf16, tag="sab")
                nc.scalar.dma_start(
                    out=sab,
                    in_=bass.AP(
                        tensor=salin_b[pbuf], offset=0, ap=[[0, 128], [1, HW]]
                    ),
                )
                st[s].update(sab=sab)

            def stage_T(s, half=None):
                x_t, yc = st[s]["x_t"], st[s]["yc"]
                # last sample's stores ride the tail-idle scalar queue
                oeng = nc.scalar if s == SPC - 1 else nc.sync
                if half is None:
                    cs = slice(0, HW)
                    sab_ap = st[s]["sab"]
                else:
                    cs = slice(2048 * half, 2048 * (half + 1))
                    sab_ap = st[s][f"sab{half}"]
                for t in range(2):
                    nc.vector.tensor_mul(
                        out=yc[:, t, cs], in0=yc[:, t, cs], in1=sab_ap
                    )
                    nc.vector.tensor_add(
                        out=x_t[:, t, cs], in0=yc[:, t, cs], in1=x_t[:, t, cs]
                    )
                    oeng.dma_start(out=out_ext[s, t, :, cs], in_=x_t[:, t, cs])

            # Emission order per step, tuned per engine queue:
            # ACT: sigmoids(s-1) first, GAP(s) late, mstage(s-2)/saσ(s-3) fill
            # DVE: yscale(s-1), premax(s-2) [feeds Pool], GAP(s), fuse(s-4)
            # PE:  chconv(s-1), conv(s-3), mean(s-2)
            # SP:  loads(s), patches(s-3), salin/bcast(s-3), AR-row/avg(s-2),
            #      stores(s-4)
            stages = (
                (0, stage_L),
                (1, stage_E),
                (3, stage_P),
                (3, stage_Cv),
                (2, stage_M),
                (0, stage_Ga),
                (0, stage_Gv),
                (2, stage_A),
                (2, stage_Me),
                (4, stage_T),
                (3, stage_Sl),
                (2, stage_Mc),
            )
            LAST = SPC - 1
            chain_stages = {stage_P, stage_Cv, stage_Sl}
            for step in range(SPC + 4):
                chain_done = False
                for d, fn in stages:
                    s = step - d
                    if not (0 <= s < SPC):
                        continue
                    if s == LAST and fn in chain_stages:
                        if not chain_done:
                            for h in range(2):
                                stage_P(s, half=h)
                                stage_Cv(s, half=h)
                                stage_Sl(s, half=h)
                            chain_done = True
                    elif s == LAST and fn is stage_T:
                        fn(s, half=0)
                        fn(s, half=1)
                    else:
                        fn(s)

    nc.compile()
    return nc


def _prep_inputs(x, w_c, w_s):
    """Shard + build per-core input maps (host side, cheap)."""
    import ml_dtypes

    wc = np.asarray(w_c, dtype=np.float32).reshape(5)
    ws4 = np.asarray(w_s, dtype=np.float32).reshape(2, 5, 5)

    # banded matrix: y'[m] = sum_k y[k] * wc[k - m + 2];  GAP 1/4096 folded in
    k = np.arange(C)[:, None]
    m = np.arange(C)[None, :]
    d = k - m + 2
    bmat = np.where((d >= 0) & (d < 5), wc[np.clip(d, 0, 4)], 0.0).astype(np.float32)
    bmat = (bmat / HW).reshape(2, 128, 256)

    # conv weights vector, rows = c*25 + ky*5 + kx ; channel-mean 1/256 folded in
    wsv = ws4.copy()
    wsv[0] /= C
    wsv = wsv.reshape(50, 1).astype(np.float32)

    xs = np.asarray(x, dtype=np.float32).astype(ml_dtypes.bfloat16).reshape(
        N_CORES, SPC, 2, 128, HW
    )
    in_maps = [{"x": xs[i], "bmat": bmat, "ws": wsv} for i in range(N_CORES)]
    return in_maps


def run(x, w_c, w_s, trace=False):
    from concourse.bass_utils import run_bass_kernel_spmd

    if "nc" not in _cache:
        _cache["nc"] = _build_graph()
    nc = _cache["nc"]
    in_maps = _prep_inputs(x, w_c, w_s)
    res = run_bass_kernel_spmd(
        nc, in_maps, core_ids=list(range(N_CORES)), trace=trace
    )
    out = np.concatenate(
        [
            res.results[i]["out"].astype(np.float32).reshape(SPC, C, H, W)
            for i in range(N_CORES)
        ],
        axis=0,
    )
    return out, res


def kernel(x, w_c, w_s):
    out, _ = run(x, w_c, w_s, trace=False)
    return out.astype(np.float32)
